# revision 4
# baseline (speedup 1.0000x reference)
"""Bahdanau-attention kernel for Trainium2 (8 NeuronCores, SPMD data parallel).

Math: the reference's per-step softmax is over a singleton axis, so the
attention weights are exactly 1.0. Hence:
    context  = values.sum(axis=1)            [B, DV]
    attn     = ones(B, T, 1)
    coverage[b, t, 0] = t                    [B, T, 1]
The W1/W2/W3/V MLP cancels out of every output.

Device work: per core, reduce a [B/8, T, DV] shard of `values` over T by
streaming [128, DV] t-chunks through the PE array against a ones column
(PSUM accumulation). attn/coverage come from a tiny host const tensor and
are written out by DMA.
"""

import os
import numpy as np

B, T, DV = 32, 2048, 1024
NCORES = 8
BP = B // NCORES          # 4 batches per core
TCH = 128                 # t-chunk = SBUF partitions
NCH = T // TCH            # 16 chunks
NSPLIT = 512              # PSUM bank free-dim limit (f32)

_CACHE = {}
LAST = {}                 # exec_time_ns etc. for the test harness


def _build_nc():
    import concourse.tile as tile
    from concourse import bacc, mybir
    from contextlib import ExitStack

    f32 = mybir.dt.float32
    nc = bacc.Bacc(
        "TRN2", target_bir_lowering=False, debug=False, num_devices=NCORES
    )

    vals = nc.dram_tensor("vals", [BP, T, DV], f32, kind="ExternalInput").ap()
    consts = nc.dram_tensor("consts", [2, T], f32, kind="ExternalInput").ap()
    ctx_out = nc.dram_tensor("ctx_out", [BP, DV], f32, kind="ExternalOutput").ap()
    attn_out = nc.dram_tensor("attn_out", [BP, T, 1], f32, kind="ExternalOutput").ap()
    cov_out = nc.dram_tensor("cov_out", [BP, T, 1], f32, kind="ExternalOutput").ap()

    with tile.TileContext(nc) as tc, ExitStack() as ctx:
        cpool = ctx.enter_context(tc.tile_pool(name="const", bufs=1))
        vpool = ctx.enter_context(tc.tile_pool(name="vals", bufs=6))
        ppool = ctx.enter_context(tc.tile_pool(name="ps", bufs=4, space="PSUM"))
        opool = ctx.enter_context(tc.tile_pool(name="out", bufs=2))

        ones_t = cpool.tile([128, 1], f32)
        nc.vector.memset(ones_t[:], 1.0)

        const_t = cpool.tile([2, T], f32)
        nc.sync.dma_start(out=const_t[:], in_=consts[:])

        for b in range(BP):
            ps = [
                ppool.tile([1, NSPLIT], f32, name=f"ps{j}", tag=f"ps{j}")
                for j in range(DV // NSPLIT)
            ]
            for i in range(NCH):
                vt = vpool.tile([TCH, DV], f32)
                nc.sync.dma_start(out=vt[:], in_=vals[b, i * TCH:(i + 1) * TCH, :])
                for j in range(DV // NSPLIT):
                    nc.tensor.matmul(
                        ps[j][:],
                        ones_t[:],
                        vt[:, j * NSPLIT:(j + 1) * NSPLIT],
                        start=(i == 0),
                        stop=(i == NCH - 1),
                    )
            ot = opool.tile([1, DV], f32)
            for j in range(DV // NSPLIT):
                nc.scalar.copy(ot[:, j * NSPLIT:(j + 1) * NSPLIT], ps[j][:])
            nc.sync.dma_start(out=ctx_out[b:b + 1, :], in_=ot[0:1, :])

        for b in range(BP):
            nc.sync.dma_start(out=attn_out[b:b + 1, :, 0], in_=const_t[0:1, :])
            nc.sync.dma_start(out=cov_out[b:b + 1, :, 0], in_=const_t[1:2, :])

    nc.compile()
    return nc


def kernel(query=None, values=None, **unused_weights):
    from concourse.bass_utils import run_bass_kernel_spmd

    values = np.ascontiguousarray(np.asarray(values, dtype=np.float32))
    assert values.shape == (B, T, DV), values.shape

    if "nc" not in _CACHE:
        _CACHE["nc"] = _build_nc()
    nc = _CACHE["nc"]

    consts = np.stack(
        [np.ones(T, dtype=np.float32), np.arange(T, dtype=np.float32)]
    )
    core_ids = list(range(NCORES))
    in_maps = [
        {"vals": values[c * BP:(c + 1) * BP], "consts": consts}
        for c in core_ids
    ]

    trace = bool(int(os.environ.get("BASS_KERNEL_TRACE", "0")))
    res = run_bass_kernel_spmd(nc, in_maps, core_ids, trace=trace)
    LAST["exec_time_ns"] = res.exec_time_ns
    LAST["results"] = res

    context = np.concatenate([res.results[c]["ctx_out"] for c in core_ids], axis=0)
    attn = np.concatenate([res.results[c]["attn_out"] for c in core_ids], axis=0)
    coverage = np.concatenate([res.results[c]["cov_out"] for c in core_ids], axis=0)
    return context, attn, coverage
